# revision 4
# baseline (speedup 1.0000x reference)
"""Segment-softmax GNN attention kernel for 8 Trainium2 NeuronCores.

Math (reference): latent = leaky_relu(x @ W + b, 0.2)  -> [E, 1]
                  out = scatter_softmax(latent, index) -> [E, 1]

Design v2 (hybrid bf16/fp8 stream + multi-window scans; memory-bound):
  Host: stable-sort edges by destination segment; shard segment-aligned
  across 8 cores (6250 segments each => no cross-core reduction).
  Per core, segments are packed first-fit-decreasing into the 128 SBUF
  partitions (J = E_pad/128 slots each), so no segment crosses a
  partition boundary and the softmax needs no cross-partition
  communication.  Features are split by |W|: the 32 largest-|w|
  features ship in bf16, the other 96 in fp8-e4m3 (weights stay bf16;
  measured end-to-end rel err ~1.1e-2 vs the 2e-2 budget).  That cuts
  the HBM stream 37% vs all-bf16.  Layouts (padded position
  P = p*J + t*CPT + c, partition p, tile t, in-tile slot c):
    xhi [128, E_pad/4] bf16: 4 consecutive slots' 32 hi-features are
      stacked on the partition axis, so ONE [128,128] stationary
      matmul against a block-diagonal Whi [128,4] yields z_hi for 4
      slots at once (8 PE instructions per 4096-edge tile).
    xlo [96, E_pad] fp8: one [96,128] stationary matmul per slot
      against Wlo [96,1] accumulates z_lo into the same PSUM group.
  Dummy slots get hi-features solving z = -500 -> exp == 0.
  Device, all static APs:
    A) stream hi/lo as fused 3-tile DMAs alternating the two HW-DGE
       queues (hi triple 768KB / lo triple 1.18MB, queue roles swap
       per triple to balance bytes); ALL x dispatches hoisted ahead of
       compute on both engines (in-order engines; pool semaphores
       throttle).  Per tile: 8 stacked hi-matmuls + 32 fp8 lo-matmuls
       -> z in PSUM; DVE leaky = max(z, 0.2z) (b==0 fast path, else
       two ops); scalar-engine Exp -> e kept in SBUF f32.
    B) segment denominators in WINDOWS of ~5 tiles, each fired as soon
       as Exp covers window_end + HSL slots: forward within-segment
       prefix scan and reversed max-carry scan over the +-HSL-padded
       slot window (masks are ONE fp8 [128, J+1] array; notend is the
       notstart view shifted by one column), then
       reciprocal_approx_fast and out = e * recip on the window body.
       Only the last window's scans (~104 slots) run after the stream.
    C) out is bf16 (host converts); 3 chunked output DMAs, the first
       two overlap the stream.
  Host: inverse-permute device output back to edge order.
"""

import os
import sys

sys.path.insert(0, "/opt/trn_rl_repo")

import numpy as np
import ml_dtypes

BF16 = ml_dtypes.bfloat16
FP8 = ml_dtypes.float8_e4m3

N_NODES = 50000
N_CORES = 8
SEG_PER_CORE = N_NODES // N_CORES          # 6250
D = 128
HI_F = 32                                  # features kept in bf16
LO_F = D - HI_F                            # features in fp8
EDGE_TILE = 4096                           # edges per phase-A tile
CPT = EDGE_TILE // 128                     # 32 slots per partition per tile
NEG_SLOPE = 0.2
HSL = 40                                   # window overlap in slots (>= max seg)
DUMMY_Z = -500.0                           # dummy-edge logit target

_compiled_cache = {}


def _win_tiles(n_xt):
    """Window sizes in tiles; last window >= 3 tiles so the previous one
    triggers before the final tile."""
    wins = []
    rem = n_xt
    while rem > 3:
        w = min(5, rem - 3)
        wins.append(w)
        rem -= w
    wins.append(rem)
    return wins


def _build_graph(E_pad: int, b_zero: bool):
    import concourse.bacc as bacc
    import concourse.tile as tile
    from concourse import bass, mybir

    f32 = mybir.dt.float32
    bf16 = mybir.dt.bfloat16
    fp8 = mybir.dt.float8e4
    n_xt = E_pad // EDGE_TILE
    J = E_pad // 128                       # slots per partition

    nc = bacc.Bacc("TRN2", target_bir_lowering=False, debug=False,
                   num_devices=N_CORES)

    xhi_d = nc.dram_tensor("xhi", [128, E_pad // 4], bf16, kind="ExternalInput")
    xlo_d = nc.dram_tensor("xlo", [LO_F, E_pad], fp8, kind="ExternalInput")
    wh4_d = nc.dram_tensor("wh4", [128, 4], bf16, kind="ExternalInput")
    wl_d = nc.dram_tensor("wl", [LO_F, 1], bf16, kind="ExternalInput")
    c02_d = nc.dram_tensor("c02", [1, 1], f32, kind="ExternalInput")
    b_d = nc.dram_tensor("bvec", [1, 1], f32, kind="ExternalInput")
    b02_d = nc.dram_tensor("b02", [1, 1], f32, kind="ExternalInput")
    nm_d = nc.dram_tensor("nmask", [128, J + 1], fp8, kind="ExternalInput")
    out_d = nc.dram_tensor("out", [E_pad, 1], bf16, kind="ExternalOutput")

    AP = bass.AP
    ALU = mybir.AluOpType
    ACT = mybir.ActivationFunctionType

    def rev(ap):
        """Reversed-free-dim view of a [128, F] AP."""
        (sp, np_), (sf, nf) = ap.ap
        return AP(tensor=ap.tensor, offset=ap.offset + sf * (nf - 1),
                  ap=[[sp, np_], [-sf, nf]])

    HCOL = 8 * 128                         # xhi cols per tile
    LCOL = EDGE_TILE                       # xlo cols per tile

    wins = _win_tiles(n_xt)
    # window w: slots [wb[w]*CPT, wb[w+1]*CPT); trigger after tile trig[w]-1
    wb = [0]
    for w in wins:
        wb.append(wb[-1] + w)
    trig = []
    for i in range(len(wins)):
        W1 = wb[i + 1] * CPT
        trig.append(min(n_xt, -(-(W1 + HSL) // CPT)))
    # output chunks: group windows into ~10-tile chunks (slot bounds)
    chunks = []
    acc = 0
    start = 0
    for i in range(len(wins)):
        acc += wins[i]
        if acc >= 10 or i == len(wins) - 1:
            chunks.append((start * CPT, wb[i + 1] * CPT, i))
            start = wb[i + 1]
            acc = 0

    with tile.TileContext(nc) as tc:
        with (
            tc.tile_pool(name="consts", bufs=1) as consts,
            tc.tile_pool(name="xhip", bufs=6) as xhip,
            tc.tile_pool(name="xlop", bufs=6) as xlop,
            tc.tile_pool(name="small", bufs=3) as small,
            tc.tile_pool(name="keep", bufs=1) as keep,
            tc.tile_pool(name="bwork", bufs=1) as bwork,
            tc.tile_pool(name="zp", bufs=2, space="PSUM") as zp,
        ):
            # --- constants: tiny, on the HW queues ahead of x ---
            wh4 = consts.tile([128, 4], bf16)
            nc.sync.dma_start(out=wh4[:], in_=wh4_d[:, :])
            wl = consts.tile([LO_F, 1], bf16)
            nc.scalar.dma_start(out=wl[:], in_=wl_d[:, :])
            bb = consts.tile([128, 1], f32)
            nc.scalar.dma_start(
                out=bb[:], in_=AP(tensor=b_d, offset=0, ap=[[0, 128], [1, 1]])
            )
            bb02 = consts.tile([128, 1], f32)
            nc.scalar.dma_start(
                out=bb02[:],
                in_=AP(tensor=b02_d, offset=0, ap=[[0, 128], [1, 1]]),
            )
            nm = consts.tile([128, J + 1], fp8)
            nc.scalar.dma_start(out=nm[:], in_=nm_d[:, :])
            nsm = nm[:, 0:J]
            nem = nm[:, 1:J + 1]

            e4_sb = keep.tile([128, J], f32)       # exp values, SBUF-resident
            out_sb = keep.tile([128, J], bf16)

            # --- phase A dispatches, ALL hoisted.  Triples alternate
            # queues; hi and lo of one triple ride different queues and
            # the roles swap per triple to balance bytes. ---
            ntri = n_xt // 3
            nsolo = n_xt % 3
            qmap = [nc.sync, nc.scalar]
            hi_bufs = []
            lo_bufs = []
            for k in range(ntri):
                qh = qmap[k % 2]
                ql = qmap[(k + 1) % 2]
                ht = xhip.tile([128, 3 * HCOL], bf16)
                qh.dma_start(
                    out=ht[:],
                    in_=AP(tensor=xhi_d, offset=k * 3 * HCOL,
                           ap=[[E_pad // 4, 128], [1, 3 * HCOL]]),
                )
                lt_ = xlop.tile([LO_F, 3 * LCOL], fp8)
                ql.dma_start(
                    out=lt_[:],
                    in_=AP(tensor=xlo_d, offset=k * 3 * LCOL,
                           ap=[[E_pad, LO_F], [1, 3 * LCOL]]),
                )
                for t in range(3):
                    hi_bufs.append(ht[:, t * HCOL:(t + 1) * HCOL])
                    lo_bufs.append(lt_[:, t * LCOL:(t + 1) * LCOL])
            for s in range(nsolo):
                i = 3 * ntri + s
                ht = xhip.tile([128, HCOL], bf16, tag="hsolo")
                qmap[s % 2].dma_start(
                    out=ht[:],
                    in_=AP(tensor=xhi_d, offset=i * HCOL,
                           ap=[[E_pad // 4, 128], [1, HCOL]]),
                )
                lt_ = xlop.tile([LO_F, LCOL], fp8, tag="lsolo")
                qmap[(s + 1) % 2].dma_start(
                    out=lt_[:],
                    in_=AP(tensor=xlo_d, offset=i * LCOL,
                           ap=[[E_pad, LO_F], [1, LCOL]]),
                )
                hi_bufs.append(ht[:])
                lo_bufs.append(lt_[:])

            def seg_denom(i):
                """Window i: scans over the padded slot window, recip +
                out = e*recip on the body."""
                d0, d1 = wb[i] * CPT, wb[i + 1] * CPT
                w0, w1 = max(0, d0 - HSL), min(J, d1 + HSL)
                wn = w1 - w0
                fwd = bwork.tile([128, wn], f32, tag=f"f{i}")
                nc.vector.tensor_tensor_scan(
                    out=fwd[:], data0=nsm[:, w0:w1], data1=e4_sb[:, w0:w1],
                    initial=0.0, op0=ALU.mult, op1=ALU.add)
                d4 = bwork.tile([128, wn], f32, tag=f"d{i}")
                nc.vector.tensor_tensor_scan(
                    out=rev(d4[:]), data0=rev(nem[:, w0:w1]),
                    data1=rev(fwd[:]), initial=0.0,
                    op0=ALU.mult, op1=ALU.max)
                dn = d1 - d0
                r4 = bwork.tile([128, dn], f32, tag=f"r{i}")
                nc.vector.reciprocal_approx_fast(out=r4[:],
                                                 in_=d4[:, d0 - w0:d1 - w0])
                nc.vector.tensor_tensor(out=out_sb[:, d0:d1],
                                        in0=e4_sb[:, d0:d1],
                                        in1=r4[:], op=ALU.mult)

            # --- compute, with windows and output chunks interleaved ---
            wi = 0
            ci = 0
            for t in range(n_xt):
                zt = zp.tile([128, CPT], f32, tag="z")
                hb = hi_bufs[t]
                lb = lo_bufs[t]
                for j in range(CPT // 4):
                    nc.tensor.matmul(zt[:, 4 * j:4 * j + 4],
                                     hb[:, 128 * j:128 * (j + 1)],
                                     wh4[:], start=True, stop=False,
                                     skip_group_check=True)
                    for i2 in range(4):
                        c = 4 * j + i2
                        nc.tensor.matmul(zt[:, c:c + 1],
                                         lb[:, 128 * c:128 * (c + 1)],
                                         wl[:], start=False, stop=(i2 == 3),
                                         skip_group_check=True)
                # leaky = max(z + b, 0.2*z + 0.2*b); one PSUM operand per op
                ut = small.tile([128, CPT], f32, tag="ut")
                nc.vector.tensor_scalar(out=ut[:], in0=zt[:],
                                        scalar1=NEG_SLOPE,
                                        scalar2=bb02[:, 0:1],
                                        op0=ALU.mult, op1=ALU.add)
                lt = small.tile([128, CPT], f32, tag="lt")
                nc.vector.scalar_tensor_tensor(
                    out=lt[:], in0=zt[:], scalar=bb[:, 0:1], in1=ut[:],
                    op0=ALU.add, op1=ALU.max)
                nc.scalar.activation(out=e4_sb[:, t * CPT:(t + 1) * CPT],
                                     in_=lt[:], func=ACT.Exp)
                while wi < len(wins) and trig[wi] == t + 1:
                    seg_denom(wi)
                    wi += 1
                    while ci < len(chunks) and chunks[ci][2] == wi - 1:
                        d0, d1, _ = chunks[ci]
                        nc.sync.dma_start(
                            out=AP(tensor=out_d, offset=d0,
                                   ap=[[J, 128], [1, d1 - d0]]),
                            in_=out_sb[:, d0:d1],
                        )
                        ci += 1
            # any windows that trigger only after the final tile
            while wi < len(wins):
                seg_denom(wi)
                wi += 1
                while ci < len(chunks) and chunks[ci][2] == wi - 1:
                    d0, d1, _ = chunks[ci]
                    nc.sync.dma_start(
                        out=AP(tensor=out_d, offset=d0,
                               ap=[[J, 128], [1, d1 - d0]]),
                        in_=out_sb[:, d0:d1],
                    )
                    ci += 1

    nc.compile()
    return nc


def _host_prep(x, W, b, index):
    """Sort/pad/bin-pack/shard on host; per-core in_maps + reassembly info."""
    x = np.ascontiguousarray(np.asarray(x, dtype=np.float32))
    W = np.asarray(W, dtype=np.float32).reshape(D)
    b = np.asarray(b, dtype=np.float32).reshape(1)
    idx = np.asarray(index).astype(np.int64).ravel()
    E = idx.shape[0]

    order = np.argsort(idx, kind="stable")
    idx_s = idx[order]
    counts = np.bincount(idx_s, minlength=N_NODES).astype(np.int64)
    seg_starts = np.zeros(N_NODES + 1, dtype=np.int64)
    np.cumsum(counts, out=seg_starts[1:])

    core_e = seg_starts[np.arange(N_CORES + 1) * SEG_PER_CORE]

    # the windowed scans assume every segment spans <= HSL slots
    assert int(counts.max()) <= HSL, f"segment length {counts.max()} > {HSL}"

    # per-core first-fit-decreasing packing of segments (no padding)
    # into 128 partitions of J slots; J grows in EDGE_TILE/128 steps
    J = 800
    packs = None
    while True:
        packs = []
        ok = True
        for k in range(N_CORES):
            s0 = k * SEG_PER_CORE
            pl = counts[s0:s0 + SEG_PER_CORE]
            sord = np.argsort(pl, kind="stable")[::-1]     # big first
            binid = np.empty(SEG_PER_CORE, dtype=np.int64)
            off = np.empty(SEG_PER_CORE, dtype=np.int64)
            rem = np.full(128, J, dtype=np.int64)
            for s in sord:
                L = int(pl[s])
                bi = int(np.argmax(rem >= L))
                if rem[bi] < L:
                    ok = False
                    break
                binid[s] = bi
                off[s] = J - rem[bi]
                rem[bi] -= L
            if not ok:
                break
            packs.append((binid, off))
        if ok:
            break
        J += EDGE_TILE // 128  # keep E_pad % EDGE_TILE == 0

    E_pad = 128 * J
    x_sorted = x[order]

    # feature split by |W|
    ford = np.argsort(-np.abs(W), kind="stable")
    hi_f, lo_f = ford[:HI_F], ford[HI_F:]
    wh, wlv = W[hi_f], W[lo_f]
    wh4 = np.zeros((128, 4), dtype=BF16)
    for j in range(4):
        wh4[HI_F * j:HI_F * (j + 1), j] = wh.astype(BF16)
    wlcol = wlv.reshape(LO_F, 1).astype(BF16)
    bvec = b.reshape(1, 1).astype(np.float32)
    b02 = (NEG_SLOPE * b).reshape(1, 1).astype(np.float32)
    c02 = np.full((1, 1), NEG_SLOPE, dtype=np.float32)
    whsq = float(wh @ wh)
    dummy_hi = ((DUMMY_Z / max(whsq, 1e-30)) * wh).astype(BF16)  # z ~ -500

    # padded position P = p*J + t*CPT + c
    #   xlo column for P:            128*(t*CPT + c) + p
    #   xhi column for P (4-stack):  128*(t*CPT + c)//4 ... see below
    Pv = np.arange(E_pad, dtype=np.int64)
    p_of = Pv // J
    s_of = Pv % J                              # slot = t*CPT + c
    locol = 128 * s_of + p_of
    hicol = 128 * (s_of // 4) * 4 // 4 * 1
    hicol = 128 * (s_of // 4) + p_of           # column block per 4-slot group
    histk = s_of % 4                           # which 32-row stack

    in_maps = []
    reasm = []
    for k in range(N_CORES):
        e0, e1 = int(core_e[k]), int(core_e[k + 1])
        cnt = e1 - e0
        s0 = k * SEG_PER_CORE
        binid, off = packs[k]
        sstart = seg_starts[s0:s0 + SEG_PER_CORE] - e0     # compact local starts

        seg_local = (idx_s[e0:e1] - s0).astype(np.int64)
        pos_in_seg = np.arange(cnt, dtype=np.int64) - sstart[seg_local]
        ppos = binid[seg_local] * J + off[seg_local] + pos_in_seg

        # hi: [128, E_pad/4] bf16, default dummy; 4-slot stacking
        xhi = np.empty((128, E_pad // 4), dtype=BF16)
        for j in range(4):
            xhi[HI_F * j:HI_F * (j + 1), :] = dummy_hi[:, None]
        xh_e = x_sorted[e0:e1][:, hi_f].astype(BF16)       # [cnt, 32]
        cols = hicol[ppos]
        stks = histk[ppos]
        for j in range(4):
            m = stks == j
            xhi[HI_F * j:HI_F * (j + 1), cols[m]] = xh_e[m].T

        # lo: [96, E_pad] fp8, default 0
        xlo = np.zeros((LO_F, E_pad), dtype=FP8)
        xlo[:, locol[ppos]] = x_sorted[e0:e1][:, lo_f].astype(FP8).T

        # per-slot segment id (unique ids for dummy slots)
        sseg = np.full(128 * J, -1, dtype=np.int64)
        pl = counts[s0:s0 + SEG_PER_CORE]
        slot0 = binid * J + off
        rep_seg = np.repeat(np.arange(SEG_PER_CORE), pl)
        rep_slot = np.repeat(slot0, pl) + (
            np.arange(int(pl.sum()), dtype=np.int64)
            - np.repeat(np.cumsum(pl) - pl, pl))
        sseg[rep_slot] = rep_seg
        dummy_mask = sseg < 0
        sseg[dummy_mask] = SEG_PER_CORE + np.arange(int(dummy_mask.sum()))
        sseg2 = sseg.reshape(128, J)
        # nmask[:, s] = notstart[s] for s in [0,J); col J = 0.
        # notend view = nmask[:, 1:J+1] (notend[s] == notstart[s+1]).
        nmask = np.zeros((128, J + 1), np.float32)
        nmask[:, 1:J] = (sseg2[:, 1:] == sseg2[:, :-1])

        in_maps.append({
            "xhi": xhi, "xlo": xlo, "wh4": wh4, "wl": wlcol,
            "c02": c02, "bvec": bvec, "b02": b02,
            "nmask": nmask.astype(FP8),
        })
        reasm.append(ppos)

    b_zero = float(b[0]) == 0.0
    return in_maps, reasm, order, core_e, E_pad, E, b_zero, float(b[0])


def _emulate_core(m, E_pad, b_val):
    """Numpy emulation of the device graph for one core."""
    J = E_pad // 128
    n_xt = E_pad // EDGE_TILE
    xhi = m["xhi"].astype(np.float32)
    xlo = m["xlo"].astype(np.float32)
    wh4 = m["wh4"].astype(np.float32)
    wl = m["wl"].astype(np.float32)
    nsm = m["nmask"].astype(np.float32)[:, 0:J]
    nem = m["nmask"].astype(np.float32)[:, 1:J + 1]

    # z per padded position
    z = np.empty(128 * J, dtype=np.float32)
    Pv = np.arange(128 * J, dtype=np.int64)
    p_of = Pv // J
    s_of = Pv % J
    # z_hi: group g = s//4, stack j = s%4
    zh_cols = xhi.T @ wh4                      # [E_pad/4, 4]
    z = zh_cols[128 * (s_of // 4) + p_of, s_of % 4]
    zl_cols = (xlo.T @ wl).ravel()             # [E_pad]
    z = z + zl_cols[128 * s_of + p_of]
    v = z + b_val
    l = np.where(v >= 0, v, NEG_SLOPE * v)
    e = np.exp(l).astype(np.float32).reshape(128, J)

    wins = _win_tiles(n_xt)
    wb = [0]
    for w in wins:
        wb.append(wb[-1] + w)
    out = np.zeros((128, J), np.float32)
    for i in range(len(wins)):
        d0, d1 = wb[i] * CPT, wb[i + 1] * CPT
        w0, w1 = max(0, d0 - HSL), min(J, d1 + HSL)
        fwd = np.zeros((128, w1 - w0), np.float32)
        st = np.zeros(128, np.float32)
        for tt in range(w1 - w0):
            st = nsm[:, w0 + tt] * st + e[:, w0 + tt]
            fwd[:, tt] = st
        d4 = np.zeros((128, w1 - w0), np.float32)
        st = np.zeros(128, np.float32)
        for tt in range(w1 - w0 - 1, -1, -1):
            st = np.maximum(nem[:, w0 + tt] * st, fwd[:, tt])
            d4[:, tt] = st
        den = d4[:, d0 - w0:d1 - w0]
        with np.errstate(divide="ignore", invalid="ignore"):
            out[:, d0:d1] = e[:, d0:d1] / den
    return out.reshape(-1).astype(BF16)


LAST_RESULTS = None  # BassKernelResults from the most recent run


def kernel(x, W, b, index):
    global LAST_RESULTS
    in_maps, reasm, order, core_e, E_pad, E, b_zero, b_val = _host_prep(
        x, W, b, index)

    if os.environ.get("KERNEL_EMULATE"):
        outs = [_emulate_core(m, E_pad, b_val) for m in in_maps]
    else:
        from concourse.bass_utils import run_bass_kernel_spmd

        key = (E_pad, b_zero)
        if key not in _compiled_cache:
            _compiled_cache[key] = _build_graph(E_pad, b_zero)
        nc = _compiled_cache[key]
        trace = bool(os.environ.get("BASS_TRACE"))
        LAST_RESULTS = run_bass_kernel_spmd(
            nc, in_maps, list(range(N_CORES)), trace=trace,
        )
        outs = [r["out"] for r in LAST_RESULTS.results]

    out_sorted = np.empty(E, dtype=np.float32)
    for k in range(N_CORES):
        e0, e1 = int(core_e[k]), int(core_e[k + 1])
        out_sorted[e0:e1] = (
            np.asarray(outs[k]).ravel().astype(np.float32)[reasm[k]])
    out = np.empty(E, dtype=np.float32)
    out[order] = out_sorted
    return out[:, None]


# revision 7
# speedup vs baseline: 1.3097x; 1.3097x over previous
"""Segment-softmax GNN attention kernel for 8 Trainium2 NeuronCores.

Math (reference): latent = leaky_relu(x @ W + b, 0.2)  -> [E, 1]
                  out = scatter_softmax(latent, index) -> [E, 1]

Design v3 (hybrid bf16/fp8 stream + multi-window scans; memory-bound):
  Host: stable-sort edges by destination segment; shard segment-aligned
  across 8 cores (6250 segments each => no cross-core reduction).
  Per core, segments are packed first-fit-decreasing into the 128 SBUF
  partitions (J = E_pad/128 slots each), so no segment crosses a
  partition boundary and the softmax needs no cross-partition
  communication.  Features are split by |W|: the 64 largest-|w|
  features ship in bf16, the other 64 in fp8-e4m3 (weights stay bf16;
  measured end-to-end rel err ~5.9e-3 vs the 2e-2 budget) -- a 25%
  HBM-stream cut vs all-bf16 at full DMA efficiency.
  Layout: slots pair up (pair q = slot 2q, 2q+1); a pair's 64 hi
  features are stacked on the partition axis (rows 0-63 = slot 2q,
  64-127 = slot 2q+1), so ONE [128,128] stationary matmul against a
  block-diagonal Whi [128,2] yields z_hi for both slots; same for the
  fp8 lo half accumulating into the same PSUM group (64 PE instructions
  per 4096-edge tile, the all-bf16 rate).  hi and lo bytes CONCATENATE
  per partition line in one uint8 DRAM tensor (per tile: 4KB hi + 2KB
  lo = 6KB lines; fused 3-tile transfers = 18KB lines, the
  DMA-efficient shape), and the SBUF tile is viewed via dtype bitcasts.
  Dummy slots get hi-features solving z = -500 -> exp == 0.
  Device, all static APs:
    A) stream triples as single 2.36MB DMAs strictly alternating the
       two HW-DGE queues; ALL x dispatches hoisted ahead of compute
       (in-order engines; pool semaphores throttle).  Per tile: 16
       hi + 16 lo stacked matmuls -> z in PSUM; DVE leaky
       (max(z+b, .2z+.2b)); scalar-engine Exp -> e in SBUF f32.
    B) segment denominators in WINDOWS of ~5 tiles, each fired as soon
       as Exp covers window_end + HSL slots: forward within-segment
       prefix scan and reversed max-carry scan over the +-HSL-padded
       slot window (masks are ONE fp8 [128, J+1] array; notend is the
       notstart view shifted by one column), then
       reciprocal_approx_fast and out = e * recip on the window body.
       Only the last window's scans (~104 slots) run after the stream.
    C) out is bf16 (host converts); 3 chunked output DMAs, the first
       two overlap the stream.
  Host: inverse-permute device output back to edge order.
"""

import os
import sys

sys.path.insert(0, "/opt/trn_rl_repo")

import numpy as np
import ml_dtypes

BF16 = ml_dtypes.bfloat16
FP8 = ml_dtypes.float8_e4m3

N_NODES = 50000
N_CORES = 8
SEG_PER_CORE = N_NODES // N_CORES          # 6250
D = 128
HI_F = 64                                  # features kept in bf16
LO_F = D - HI_F                            # features in fp8
EDGE_TILE = 4096                           # edges per phase-A tile
CPT = EDGE_TILE // 128                     # 32 slots per partition per tile
PPT = CPT // 2                             # 16 slot-pairs per tile
TBYTES = PPT * 128 * (2 * 2 + 1 * 2) // 2  # bytes per partition per tile
TBYTES = PPT * 128 * 2 + PPT * 128         # 4096 hi + 2048 lo = 6144
NEG_SLOPE = 0.2
HSL = 40                                   # window overlap in slots (>= max seg)
DUMMY_Z = -500.0                           # dummy-edge logit target

_compiled_cache = {}


def _win_tiles(n_xt):
    """Window sizes in tiles; last window >= 3 tiles so the previous one
    triggers before the final tile."""
    wins = []
    rem = n_xt
    while rem > 3:
        w = min(5, rem - 3)
        wins.append(w)
        rem -= w
    wins.append(rem)
    return wins


def _build_graph(E_pad: int):
    import concourse.bacc as bacc
    import concourse.tile as tile
    from concourse import bass, mybir

    f32 = mybir.dt.float32
    bf16 = mybir.dt.bfloat16
    fp8 = mybir.dt.float8e4
    u8 = mybir.dt.uint8
    n_xt = E_pad // EDGE_TILE
    J = E_pad // 128                       # slots per partition

    nc = bacc.Bacc("TRN2", target_bir_lowering=False, debug=False,
                   num_devices=N_CORES)

    xmix_d = nc.dram_tensor("xmix", [128, n_xt * TBYTES], u8,
                            kind="ExternalInput")
    wh2_d = nc.dram_tensor("wh2", [128, 2], bf16, kind="ExternalInput")
    wl2_d = nc.dram_tensor("wl2", [128, 2], bf16, kind="ExternalInput")
    b_d = nc.dram_tensor("bvec", [1, 1], f32, kind="ExternalInput")
    b02_d = nc.dram_tensor("b02", [1, 1], f32, kind="ExternalInput")
    nm_d = nc.dram_tensor("nmask", [128, J + 1], fp8, kind="ExternalInput")
    out_d = nc.dram_tensor("out", [E_pad, 1], bf16, kind="ExternalOutput")

    AP = bass.AP
    ALU = mybir.AluOpType
    ACT = mybir.ActivationFunctionType

    def rev(ap):
        """Reversed-free-dim view of a [128, F] AP."""
        (sp, np_), (sf, nf) = ap.ap
        return AP(tensor=ap.tensor, offset=ap.offset + sf * (nf - 1),
                  ap=[[sp, np_], [-sf, nf]])

    wins = _win_tiles(n_xt)
    wb = [0]
    for w in wins:
        wb.append(wb[-1] + w)
    trig = [min(n_xt, -(-(wb[i + 1] * CPT + HSL) // CPT))
            for i in range(len(wins))]
    # output chunks: group windows into ~10-tile chunks
    chunks = []
    acc = 0
    start = 0
    for i in range(len(wins)):
        acc += wins[i]
        if acc >= 10 or i == len(wins) - 1:
            chunks.append((start * CPT, wb[i + 1] * CPT, i))
            start = wb[i + 1]
            acc = 0

    with tile.TileContext(nc) as tc:
        with (
            tc.tile_pool(name="consts", bufs=1) as consts,
            tc.tile_pool(name="xin", bufs=6) as xin,
            tc.tile_pool(name="small", bufs=3) as small,
            tc.tile_pool(name="keep", bufs=1) as keep,
            tc.tile_pool(name="bwork", bufs=1) as bwork,
            tc.tile_pool(name="zp", bufs=2, space="PSUM") as zp,
        ):
            # --- constants: tiny, on the HW queues ahead of x ---
            wh2 = consts.tile([128, 2], bf16)
            nc.sync.dma_start(out=wh2[:], in_=wh2_d[:, :])
            wl2 = consts.tile([128, 2], bf16)
            nc.scalar.dma_start(out=wl2[:], in_=wl2_d[:, :])
            bb = consts.tile([128, 1], f32)
            nc.scalar.dma_start(
                out=bb[:], in_=AP(tensor=b_d, offset=0, ap=[[0, 128], [1, 1]])
            )
            bb02 = consts.tile([128, 1], f32)
            nc.scalar.dma_start(
                out=bb02[:],
                in_=AP(tensor=b02_d, offset=0, ap=[[0, 128], [1, 1]]),
            )
            nm = consts.tile([128, J + 1], fp8)
            nc.scalar.dma_start(out=nm[:], in_=nm_d[:, :])
            nsm = nm[:, 0:J]
            nem = nm[:, 1:J + 1]

            e4_sb = keep.tile([128, J], f32)       # exp values, SBUF-resident
            out_sb = keep.tile([128, J], bf16)

            # --- phase A dispatches, ALL hoisted; strict alternation ---
            ntri = n_xt // 3
            nsolo = n_xt % 3
            qmap = [nc.sync, nc.scalar]
            qbytes = [0, 0]
            views = []                             # per tile: (hi AP, lo AP)
            for k in range(ntri):
                q = k % 2
                qbytes[q] += 3 * TBYTES
                xt = xin.tile([128, 3 * TBYTES], u8)
                qmap[q].dma_start(
                    out=xt[:],
                    in_=AP(tensor=xmix_d, offset=k * 3 * TBYTES,
                           ap=[[n_xt * TBYTES, 128], [1, 3 * TBYTES]]),
                )
                for t in range(3):
                    hi = xt[:, t * TBYTES:t * TBYTES + 2 * PPT * 128].bitcast(
                        bf16)
                    lo = xt[:, t * TBYTES + 2 * PPT * 128:
                            (t + 1) * TBYTES].bitcast(fp8)
                    views.append((hi, lo))
            for s in range(nsolo):
                i = 3 * ntri + s
                q = 0 if qbytes[0] <= qbytes[1] else 1
                qbytes[q] += TBYTES
                xt = xin.tile([128, TBYTES], u8, tag="solo")
                qmap[q].dma_start(
                    out=xt[:],
                    in_=AP(tensor=xmix_d, offset=i * TBYTES,
                           ap=[[n_xt * TBYTES, 128], [1, TBYTES]]),
                )
                hi = xt[:, 0:2 * PPT * 128].bitcast(bf16)
                lo = xt[:, 2 * PPT * 128:TBYTES].bitcast(fp8)
                views.append((hi, lo))

            def seg_denom(i):
                """Window i: scans over the padded slot window, recip +
                out = e*recip on the body."""
                d0, d1 = wb[i] * CPT, wb[i + 1] * CPT
                w0, w1 = max(0, d0 - HSL), min(J, d1 + HSL)
                wn = w1 - w0
                fwd = bwork.tile([128, wn], f32, tag=f"f{i}")
                nc.vector.tensor_tensor_scan(
                    out=fwd[:], data0=nsm[:, w0:w1], data1=e4_sb[:, w0:w1],
                    initial=0.0, op0=ALU.mult, op1=ALU.add)
                d4 = bwork.tile([128, wn], f32, tag=f"d{i}")
                nc.vector.tensor_tensor_scan(
                    out=rev(d4[:]), data0=rev(nem[:, w0:w1]),
                    data1=rev(fwd[:]), initial=0.0,
                    op0=ALU.mult, op1=ALU.max)
                dn = d1 - d0
                r4 = bwork.tile([128, dn], f32, tag=f"r{i}")
                nc.vector.reciprocal_approx_fast(out=r4[:],
                                                 in_=d4[:, d0 - w0:d1 - w0])
                nc.vector.tensor_tensor(out=out_sb[:, d0:d1],
                                        in0=e4_sb[:, d0:d1],
                                        in1=r4[:], op=ALU.mult)

            def out_chunk(ci):
                d0, d1, _ = chunks[ci]
                nc.sync.dma_start(
                    out=AP(tensor=out_d, offset=d0,
                           ap=[[J, 128], [1, d1 - d0]]),
                    in_=out_sb[:, d0:d1],
                )

            # --- compute, with windows and output chunks interleaved ---
            wi = 0
            ci = 0
            for t in range(n_xt):
                zt = zp.tile([128, CPT], f32, tag="z")
                hi, lo = views[t]
                for j in range(PPT):
                    nc.tensor.matmul(zt[:, 2 * j:2 * j + 2],
                                     hi[:, 128 * j:128 * (j + 1)],
                                     wh2[:], start=True, stop=False,
                                     skip_group_check=True)
                    nc.tensor.matmul(zt[:, 2 * j:2 * j + 2],
                                     lo[:, 128 * j:128 * (j + 1)],
                                     wl2[:], start=False, stop=True,
                                     skip_group_check=True)
                # leaky = max(z + b, 0.2*z + 0.2*b); one PSUM operand per op
                ut = small.tile([128, CPT], f32, tag="ut")
                nc.vector.tensor_scalar(out=ut[:], in0=zt[:],
                                        scalar1=NEG_SLOPE,
                                        scalar2=bb02[:, 0:1],
                                        op0=ALU.mult, op1=ALU.add)
                lt = small.tile([128, CPT], f32, tag="lt")
                nc.vector.scalar_tensor_tensor(
                    out=lt[:], in0=zt[:], scalar=bb[:, 0:1], in1=ut[:],
                    op0=ALU.add, op1=ALU.max)
                nc.scalar.activation(out=e4_sb[:, t * CPT:(t + 1) * CPT],
                                     in_=lt[:], func=ACT.Exp)
                while wi < len(wins) and trig[wi] == t + 1:
                    seg_denom(wi)
                    wi += 1
                    while ci < len(chunks) and chunks[ci][2] == wi - 1:
                        out_chunk(ci)
                        ci += 1
            while wi < len(wins):
                seg_denom(wi)
                wi += 1
                while ci < len(chunks) and chunks[ci][2] == wi - 1:
                    out_chunk(ci)
                    ci += 1

    nc.compile()
    return nc


def _host_prep(x, W, b, index):
    """Sort/pad/bin-pack/shard on host; per-core in_maps + reassembly info."""
    x = np.ascontiguousarray(np.asarray(x, dtype=np.float32))
    W = np.asarray(W, dtype=np.float32).reshape(D)
    b = np.asarray(b, dtype=np.float32).reshape(1)
    idx = np.asarray(index).astype(np.int64).ravel()
    E = idx.shape[0]

    order = np.argsort(idx, kind="stable")
    idx_s = idx[order]
    counts = np.bincount(idx_s, minlength=N_NODES).astype(np.int64)
    seg_starts = np.zeros(N_NODES + 1, dtype=np.int64)
    np.cumsum(counts, out=seg_starts[1:])

    core_e = seg_starts[np.arange(N_CORES + 1) * SEG_PER_CORE]

    # the windowed scans assume every segment spans <= HSL slots
    assert int(counts.max()) <= HSL, f"segment length {counts.max()} > {HSL}"

    # per-core first-fit-decreasing packing of segments (no padding)
    # into 128 partitions of J slots; J grows in EDGE_TILE/128 steps
    J = 800
    packs = None
    while True:
        packs = []
        ok = True
        for k in range(N_CORES):
            s0 = k * SEG_PER_CORE
            pl = counts[s0:s0 + SEG_PER_CORE]
            sord = np.argsort(pl, kind="stable")[::-1]     # big first
            binid = np.empty(SEG_PER_CORE, dtype=np.int64)
            off = np.empty(SEG_PER_CORE, dtype=np.int64)
            rem = np.full(128, J, dtype=np.int64)
            for s in sord:
                L = int(pl[s])
                bi = int(np.argmax(rem >= L))
                if rem[bi] < L:
                    ok = False
                    break
                binid[s] = bi
                off[s] = J - rem[bi]
                rem[bi] -= L
            if not ok:
                break
            packs.append((binid, off))
        if ok:
            break
        J += EDGE_TILE // 128  # keep E_pad % EDGE_TILE == 0

    E_pad = 128 * J
    n_xt = E_pad // EDGE_TILE
    x_sorted = x[order]

    # feature split by |W|
    ford = np.argsort(-np.abs(W), kind="stable")
    hi_f, lo_f = ford[:HI_F], ford[HI_F:]
    wh, wlv = W[hi_f], W[lo_f]
    wh2 = np.zeros((128, 2), dtype=BF16)
    wl2 = np.zeros((128, 2), dtype=BF16)
    for j in range(2):
        wh2[HI_F * j:HI_F * (j + 1), j] = wh.astype(BF16)
        wl2[LO_F * j:LO_F * (j + 1), j] = wlv.astype(BF16)
    bvec = b.reshape(1, 1).astype(np.float32)
    b02 = (NEG_SLOPE * b).reshape(1, 1).astype(np.float32)
    whsq = float(wh @ wh)
    dummy_hi = ((DUMMY_Z / max(whsq, 1e-30)) * wh).astype(BF16)  # z ~ -500

    in_maps = []
    reasm = []
    for k in range(N_CORES):
        e0, e1 = int(core_e[k]), int(core_e[k + 1])
        cnt = e1 - e0
        s0 = k * SEG_PER_CORE
        binid, off = packs[k]
        sstart = seg_starts[s0:s0 + SEG_PER_CORE] - e0     # compact local starts

        seg_local = (idx_s[e0:e1] - s0).astype(np.int64)
        pos_in_seg = np.arange(cnt, dtype=np.int64) - sstart[seg_local]
        ppos = binid[seg_local] * J + off[seg_local] + pos_in_seg

        # per-(feat, padded position) values
        hi_vals = np.empty((HI_F, E_pad), dtype=BF16)
        hi_vals[:] = dummy_hi[:, None]
        hi_vals[:, ppos] = x_sorted[e0:e1][:, hi_f].astype(BF16).T
        lo_vals = np.zeros((LO_F, E_pad), dtype=FP8)
        lo_vals[:, ppos] = x_sorted[e0:e1][:, lo_f].astype(FP8).T

        # pack: row 64*j+i, tile t line = [hi pairs 4KB | lo pairs 2KB];
        # hi col 128*q+m (q = pair in tile) = hi_vals[i, m*J + 2*(16t+q)+j]
        hv = hi_vals.reshape(HI_F, 128, J)              # [i, m, s]
        lv = lo_vals.reshape(LO_F, 128, J)
        hi_part = np.concatenate(
            [hv[:, :, 0::2].transpose(0, 2, 1),         # [i, qg, m] side 0
             hv[:, :, 1::2].transpose(0, 2, 1)], axis=0)  # -> [128, J/2, 128]
        lo_part = np.concatenate(
            [lv[:, :, 0::2].transpose(0, 2, 1),
             lv[:, :, 1::2].transpose(0, 2, 1)], axis=0)
        hi_u8 = np.ascontiguousarray(hi_part).view(np.uint8).reshape(
            128, n_xt, PPT * 128 * 2)
        lo_u8 = np.ascontiguousarray(lo_part).view(np.uint8).reshape(
            128, n_xt, PPT * 128)
        xmix = np.concatenate([hi_u8, lo_u8], axis=2).reshape(
            128, n_xt * TBYTES)

        # per-slot segment id (unique ids for dummy slots)
        sseg = np.full(128 * J, -1, dtype=np.int64)
        pl = counts[s0:s0 + SEG_PER_CORE]
        slot0 = binid * J + off
        rep_seg = np.repeat(np.arange(SEG_PER_CORE), pl)
        rep_slot = np.repeat(slot0, pl) + (
            np.arange(int(pl.sum()), dtype=np.int64)
            - np.repeat(np.cumsum(pl) - pl, pl))
        sseg[rep_slot] = rep_seg
        dummy_mask = sseg < 0
        sseg[dummy_mask] = SEG_PER_CORE + np.arange(int(dummy_mask.sum()))
        sseg2 = sseg.reshape(128, J)
        # nmask[:, s] = notstart[s] for s in [0,J); col J = 0.
        # notend view = nmask[:, 1:J+1] (notend[s] == notstart[s+1]).
        nmask = np.zeros((128, J + 1), np.float32)
        nmask[:, 1:J] = (sseg2[:, 1:] == sseg2[:, :-1])

        in_maps.append({
            "xmix": xmix, "wh2": wh2, "wl2": wl2,
            "bvec": bvec, "b02": b02,
            "nmask": nmask.astype(FP8),
        })
        reasm.append(ppos)

    return in_maps, reasm, order, core_e, E_pad, E, float(b[0])


def _emulate_core(m, E_pad, b_val):
    """Numpy emulation of the device graph for one core (decodes xmix)."""
    J = E_pad // 128
    n_xt = E_pad // EDGE_TILE
    xmix = m["xmix"].reshape(128, n_xt, TBYTES)
    wh2 = m["wh2"].astype(np.float32)
    wl2 = m["wl2"].astype(np.float32)
    nsm = m["nmask"].astype(np.float32)[:, 0:J]
    nem = m["nmask"].astype(np.float32)[:, 1:J + 1]

    hi = np.ascontiguousarray(xmix[:, :, :PPT * 128 * 2]).view(BF16).reshape(
        128, n_xt, PPT, 128).astype(np.float32)          # [k, t, q, m]
    lo = np.ascontiguousarray(xmix[:, :, PPT * 128 * 2:]).view(FP8).reshape(
        128, n_xt, PPT, 128).astype(np.float32)

    # z[slot s=2*(16t+q)+j, m] = sum_k hi[k,t,q,m]*wh2[k,j] + lo..*wl2[k,j]
    zpair = np.einsum('ktqm,kj->tqjm', hi, wh2) + \
        np.einsum('ktqm,kj->tqjm', lo, wl2)              # [t, q, j, m]
    z = np.empty((128, J), np.float32)                    # [m, s]
    sidx = (2 * (np.arange(n_xt)[:, None, None] * PPT
                 + np.arange(PPT)[None, :, None])
            + np.arange(2)[None, None, :])                # [t, q, j]
    z[:, sidx.ravel()] = zpair.reshape(-1, 128).T
    v = z + b_val
    l = np.where(v >= 0, v, NEG_SLOPE * v)
    e = np.exp(l).astype(np.float32)

    wins = _win_tiles(n_xt)
    wb = [0]
    for w in wins:
        wb.append(wb[-1] + w)
    out = np.zeros((128, J), np.float32)
    for i in range(len(wins)):
        d0, d1 = wb[i] * CPT, wb[i + 1] * CPT
        w0, w1 = max(0, d0 - HSL), min(J, d1 + HSL)
        fwd = np.zeros((128, w1 - w0), np.float32)
        st = np.zeros(128, np.float32)
        for tt in range(w1 - w0):
            st = nsm[:, w0 + tt] * st + e[:, w0 + tt]
            fwd[:, tt] = st
        d4 = np.zeros((128, w1 - w0), np.float32)
        st = np.zeros(128, np.float32)
        for tt in range(w1 - w0 - 1, -1, -1):
            st = np.maximum(nem[:, w0 + tt] * st, fwd[:, tt])
            d4[:, tt] = st
        den = d4[:, d0 - w0:d1 - w0]
        with np.errstate(divide="ignore", invalid="ignore"):
            out[:, d0:d1] = e[:, d0:d1] / den
    return out.reshape(-1).astype(BF16)


LAST_RESULTS = None  # BassKernelResults from the most recent run


def kernel(x, W, b, index):
    global LAST_RESULTS
    in_maps, reasm, order, core_e, E_pad, E, b_val = _host_prep(
        x, W, b, index)

    if os.environ.get("KERNEL_EMULATE"):
        outs = [_emulate_core(m, E_pad, b_val) for m in in_maps]
    else:
        from concourse.bass_utils import run_bass_kernel_spmd

        if E_pad not in _compiled_cache:
            _compiled_cache[E_pad] = _build_graph(E_pad)
        nc = _compiled_cache[E_pad]
        trace = bool(os.environ.get("BASS_TRACE"))
        LAST_RESULTS = run_bass_kernel_spmd(
            nc, in_maps, list(range(N_CORES)), trace=trace,
        )
        outs = [r["out"] for r in LAST_RESULTS.results]

    out_sorted = np.empty(E, dtype=np.float32)
    for k in range(N_CORES):
        e0, e1 = int(core_e[k]), int(core_e[k + 1])
        out_sorted[e0:e1] = (
            np.asarray(outs[k]).ravel().astype(np.float32)[reasm[k]])
    out = np.empty(E, dtype=np.float32)
    out[order] = out_sorted
    return out[:, None]
